# revision 38
# baseline (speedup 1.0000x reference)
"""Trainium2 Bass kernel for nn_FEMREncoderLayer (RMSNorm + fused QKV + RoPE +
sliding-window local attention + parallel gelu FFN + joint output projection).

Data-parallel over 8 NeuronCores: core i handles batch i//4, tokens
[(i%4)*1024, (i%4)*1024+1024), with a 512-token halo for the local attention's
previous-block keys/values (zeros + masked out for the first block of each
batch). Q/K flow feature-major so weight tiles are the PE stationary operand;
V is produced token-major directly (no on-device transposes). Attention
scores for one (head, q-block) land in a bin-packed 5-bank PSUM region so the
softmax exp runs as two wide ACT calls; the causal/halo mask is one batched
multiply against a host-built mask tile; softmax denominators are collected
and inverted in a single batched reciprocal at the output-projection stage.
Matmul inputs are bf16 (fp32 PSUM accumulation).
"""
import numpy as np
import ml_dtypes
from contextlib import ExitStack

B, S, H, NH, HD, I, W = 2, 4096, 1024, 16, 64, 4096, 512
EPS = 1e-6
N_CORES = 8
OWN = 1024            # tokens owned per core
SHARD = OWN + W       # plus halo

bf16 = ml_dtypes.bfloat16

# stream_shuffle mask: swap adjacent partition pairs within each 32-group
_SHUF = []
for _i in range(16):
    _SHUF += [2 * _i + 1, 2 * _i]

# score-region packing: per key-chunk jt -> (offset, width) in the [128,2560]
# PSUM region. jt 0-3 = halo 128-chunks, jt 4-7 = own 128-chunks. Widths are
# the valid query ranges; offsets bin-pack into 512-f32 PSUM banks.
EXW = [128, 256, 384, 512, 512, 384, 256, 128]
EXOFF = {4: 0, 3: 512, 5: 1024, 0: 1408, 2: 1536, 7: 1920, 1: 2048, 6: 2304}
JT_HALF1 = [4, 3, 5, 0]   # fill banks 0-2 (cols 0:1536)
JT_HALF2 = [2, 7, 1, 6]   # fill banks 3-4 (cols 1536:2560)

_NC_CACHE = None


def _split_sync_waits(nc, mybir, max_waits=1):
    """This container's walrus encodes at most one sync-wait command per
    instruction; spread Tile's extra waits over preceding same-engine NoOps."""
    for f in nc.m.functions:
        for bb in f.blocks:
            out = []
            changed = False
            for ins in bb.instructions:
                si = ins.sync_info
                if si is not None and si.on_wait and len(si.on_wait) > max_waits:
                    waits = list(si.on_wait)
                    extra, keep = waits[:-max_waits], waits[-max_waits:]
                    for i, w in enumerate(extra):
                        out.append(mybir.InstNoOp(
                            name=f"{ins.name}-sw{i}", engine=ins.engine,
                            ins=[], outs=[],
                            sync_info=mybir.SyncInfo(on_wait=[w], on_update=[])))
                    si.on_wait = keep
                    changed = True
                out.append(ins)
            if changed:
                del bb.instructions[:]
                for ins in out:
                    bb.add_instruction(ins)
    return nc


def _build():
    global _NC_CACHE
    if _NC_CACHE is not None:
        return _NC_CACHE
    import concourse.bass as bass
    import concourse.tile as tile
    import concourse.mybir as mybir

    F32 = mybir.dt.float32
    BF = mybir.dt.bfloat16
    AF = mybir.ActivationFunctionType

    nc = bass.Bass()
    xbT_d = nc.dram_tensor("xbT", [H, SHARD], BF, kind="ExternalInput")
    cosT = nc.dram_tensor("cosT", [128, SHARD], F32, kind="ExternalInput")
    sinST = nc.dram_tensor("sinST", [128, SHARD], F32, kind="ExternalInput")
    wqkv_r = nc.dram_tensor("wqkv_r", [16, 128, 1024], BF, kind="ExternalInput")
    wv_r = nc.dram_tensor("wv_r", [8, 128, 1024], BF, kind="ExternalInput")
    win_r = nc.dram_tensor("win_r", [32, 128, 1024], BF, kind="ExternalInput")
    wout_r = nc.dram_tensor("wout_r", [2, 20, 128, 1024], BF, kind="ExternalInput")
    b_in_t = nc.dram_tensor("b_in_t", [128, 32], F32, kind="ExternalInput")
    b_out_t = nc.dram_tensor("b_out_t", [128, 8], F32, kind="ExternalInput")
    mask0_d = nc.dram_tensor("mask0", [128, 2560], BF, kind="ExternalInput")
    mask1_d = nc.dram_tensor("mask1", [128, 2560], BF, kind="ExternalInput")
    out_d = nc.dram_tensor("outT", [H, OWN], F32, kind="ExternalOutput")
    ff_d = nc.dram_tensor("ff_d", [32, 128, OWN], BF, kind="Internal")
    rs_d = nc.dram_tensor("rs_sc", [12, 128], F32, kind="Internal")
    rs2_d = nc.dram_tensor("rs_sc2", [12, 128], BF, kind="Internal")

    with tile.TileContext(nc) as tc, ExitStack() as top:
        consts = top.enter_context(tc.tile_pool(name="consts", bufs=1))
        masks = [consts.tile([128, 2560], BF, tag=f"mask{b}", name=f"mask{b}")
                 for b in range(2)]
        ones128 = consts.tile([128, 1], BF, tag="ones128", name="ones128")
        b_in_sb = consts.tile([128, 32], F32, tag="b_in_sb", name="b_in_sb")
        b_out_sb = consts.tile([128, 8], F32, tag="b_out_sb", name="b_out_sb")
        eps_sb = consts.tile([1, 1], F32, tag="eps_sb", name="eps_sb")
        rsB2 = consts.tile([128, SHARD], BF, tag="rsB2", name="rsB2")
        rsT = consts.tile([128, 12], F32, tag="rsT", name="rsT")
        nc.vector.memset(eps_sb[:], EPS)
        nc.vector.memset(ones128[:], 1.0)

        attnT_pool = top.enter_context(tc.tile_pool(name="attnT", bufs=1))
        attnT = [attnT_pool.tile([128, OWN], BF, tag=f"at{i}", name=f"at{i}")
                 for i in range(8)]
        den_sb = attnT_pool.tile([16, 1024], F32, tag="den_sb", name="den_sb")

        bd = ExitStack()
        with bd:
            # ---- stages A-C live inside this scope; its SBUF frees before F
            with tc.tile_pool(name="qT", bufs=1) as qT_pool, \
                 tc.tile_pool(name="kT", bufs=1) as kT_pool, \
                 tc.tile_pool(name="vb", bufs=1) as vb_pool, \
                 tc.tile_pool(name="xbo", bufs=1) as xbo_pool:
                qT = [qT_pool.tile([128, OWN], BF, tag=f"q{i}", name=f"q{i}")
                      for i in range(8)]
                kT = [kT_pool.tile([128, SHARD], BF, tag=f"k{i}", name=f"k{i}")
                      for i in range(8)]
                vbuf = [[vb_pool.tile([128, 16 * 65], BF, tag=f"vb{p}{k}",
                                      name=f"vb{p}{k}") for k in range(4)]
                        for p in range(3)]
                for p in range(3):
                    for k in range(4):
                        ap = vbuf[p][k][:, :].rearrange(
                            "p (h s) -> p h s", s=65)[:, :, 64:65]
                        nc.vector.memset(ap, 1.0)
                xbO = [xbo_pool.tile([128, OWN], BF, tag=f"xb{i}", name=f"xb{i}")
                       for i in range(8)]

                # ---- stage A+B: x load, stats, QKV+RoPE, token-major V ----
                with tc.tile_pool(name="xbh", bufs=1) as xbh_pool, \
                     tc.tile_pool(name="wvp", bufs=1) as wv_pool:
                    xbH = [xbh_pool.tile([128, W], BF, tag=f"xh{i}", name=f"xh{i}")
                           for i in range(8)]
                    wv_sb = [wv_pool.tile([128, 1024], BF, tag=f"wv{h}",
                                          name=f"wv{h}") for h in range(8)]
                    with tc.tile_pool(name="aux", bufs=1) as aux_pool, \
                         tc.tile_pool(name="sq", bufs=2) as sq_pool, \
                         tc.tile_pool(name="wq", bufs=3) as wq_pool, \
                         tc.tile_pool(name="rsc", bufs=3) as rsc_pool, \
                         tc.tile_pool(name="pms", bufs=1, space="PSUM") as pms_pool, \
                         tc.tile_pool(name="pqkv", bufs=4, space="PSUM") as pqkv_pool:
                        cosR = aux_pool.tile([128, SHARD], F32, tag="cosR", name="cosR")
                        sinR = aux_pool.tile([128, SHARD], F32, tag="sinR", name="sinR")
                        rs_tmp = aux_pool.tile([1, SHARD], F32, tag="rs_tmp",
                                               name="rs_tmp")
                        nc.sync.dma_start(cosR[:], cosT[:])
                        nc.sync.dma_start(sinR[:], sinST[:])
                        for i in range(8):
                            nc.sync.dma_start(xbH[i][:],
                                              xbT_d[128 * i:128 * (i + 1), 0:W])
                            nc.sync.dma_start(xbO[i][:],
                                              xbT_d[128 * i:128 * (i + 1), W:])
                        pms = pms_pool.tile([1, SHARD], F32, tag="pms", name="pms")
                        for i in range(8):
                            sqh = sq_pool.tile([128, W], BF, tag="sqh", name="sqh")
                            nc.scalar.square(sqh[:], xbH[i][:])
                            nc.tensor.matmul(pms[:, 0:512], ones128[:], sqh[:],
                                             start=(i == 0), stop=(i == 7))
                            sqo = sq_pool.tile([128, OWN], BF, tag="sqo", name="sqo")
                            nc.scalar.square(sqo[:], xbO[i][:])
                            for c in range(2):
                                nc.tensor.matmul(
                                    pms[:, 512 * (c + 1):512 * (c + 2)],
                                    ones128[:], sqo[:, 512 * c:512 * (c + 1)],
                                    start=(i == 0), stop=(i == 7))
                        # rs = 1/sqrt(ms/H + eps), then x *= rs in place so
                        # Q/K/V/FFN matmuls all consume the normalized x.
                        # The reciprocal runs on the [128,12] token-major view
                        # (DVE recip on 1 partition costs ~13us; on 128 it's
                        # ~0.1us). SBUF->SBUF partition-remap DMAs produce
                        # garbage, so the remaps round-trip through DRAM.
                        nc.scalar.activation(rs_tmp[:], pms[:], AF.Sqrt,
                                             bias=eps_sb[:], scale=1.0 / H)
                        nc.sync.dma_start(rs_d[:, :], rs_tmp[:, :])
                        nc.sync.dma_start(
                            rsT[:, :], rs_d[:, :].rearrange("j p -> p j"))
                        rsTb = aux_pool.tile([128, 12], BF, tag="rsTb",
                                             name="rsTb")
                        with nc.allow_low_precision(reason="rms scale to bf16"):
                            nc.vector.reciprocal(rsTb[:, :], rsT[:, :])
                        nc.sync.dma_start(
                            rs2_d[:, :].rearrange("j p -> p j"), rsTb[:, :])
                        nc.sync.dma_start(
                            rsB2[:],
                            rs2_d[:, :].rearrange("j p -> (j p)").rearrange(
                                "(o t) -> o t", o=1).to_broadcast((128, SHARD)))
                        for i in range(8):
                            nc.vector.tensor_mul(xbH[i][:], xbH[i][:],
                                                 rsB2[:, 0:W])
                            nc.vector.tensor_mul(xbO[i][:], xbO[i][:],
                                                 rsB2[:, W:SHARD])

                        # Q (m 0-7) and K (m 8-15) with RoPE fused from PSUM
                        for m in range(16):
                            is_q = m < 8
                            chunks = (1, 2) if is_q else (0, 1, 2)
                            wqm = wq_pool.tile([128, 1024], BF, tag="wqm", name="wqm")
                            nc.sync.dma_start(wqm[:, :], wqkv_r[m])
                            for c in chunks:
                                ps = pqkv_pool.tile([128, 512], F32, tag="pqkv",
                                                    name="pqkv")
                                for h in range(8):
                                    rhs = (xbH[h][:] if c == 0
                                           else xbO[h][:, 512 * (c - 1):512 * c])
                                    nc.tensor.matmul(
                                        ps[:], wqm[:, 128 * h:128 * (h + 1)],
                                        rhs, start=(h == 0), stop=(h == 7))
                                if is_q:
                                    dest = qT[m][:, 512 * (c - 1):512 * c]
                                else:
                                    dest = kT[m - 8][:, 512 * c:512 * (c + 1)]
                                qc = rsc_pool.tile([128, 512], BF, tag="qc", name="qc")
                                shf = rsc_pool.tile([128, 512], F32, tag="shf",
                                                    name="shf")
                                shs = rsc_pool.tile([128, 512], BF, tag="shs",
                                                    name="shs")
                                nc.vector.tensor_mul(
                                    qc[:], ps[:], cosR[:, 512 * c:512 * (c + 1)])
                                nc.vector.stream_shuffle(shf[:], ps[:], _SHUF)
                                nc.gpsimd.tensor_mul(
                                    shs[:], shf[:], sinR[:, 512 * c:512 * (c + 1)])
                                nc.gpsimd.tensor_add(dest, qc[:], shs[:])

                    # V: token-major (stationary = normalized x, moving = w_v)
                    with tc.tile_pool(name="pv", bufs=4, space="PSUM") as pv_pool:
                        for h in range(8):
                            nc.sync.dma_start(wv_sb[h][:], wv_r[h])
                        for T in range(12):
                            t0 = 128 * T if T < 4 else 128 * (T - 4)
                            for c in range(2):
                                psv = pv_pool.tile([128, 512], F32, tag="psv",
                                                   name="psv")
                                for h in range(8):
                                    xst = (xbH[h][:, t0:t0 + 128] if T < 4
                                           else xbO[h][:, t0:t0 + 128])
                                    nc.tensor.matmul(
                                        psv[:], xst,
                                        wv_sb[h][:, 512 * c:512 * (c + 1)],
                                        start=(h == 0), stop=(h == 7))
                                dst = vbuf[T // 4][T % 4][:, :].rearrange(
                                    "p (h s) -> p h s", s=65)[:, 8 * c:8 * c + 8, 0:64]
                                src = psv[:, :].rearrange("p (h s) -> p h s", s=64)
                                nc.vector.tensor_copy(dst, src)

                # ---- stage C: attention + FFN, interleaved ----
                with tc.tile_pool(name="exp", bufs=2) as exp_pool, \
                     tc.tile_pool(name="wi", bufs=3) as wi_pool, \
                     tc.tile_pool(name="ffs", bufs=4) as ffs_pool, \
                     tc.tile_pool(name="dent", bufs=2) as den_pool, \
                     tc.tile_pool(name="sc", bufs=1, space="PSUM") as sc_pool, \
                     tc.tile_pool(name="pa", bufs=2, space="PSUM") as pa_pool, \
                     tc.tile_pool(name="pff", bufs=1, space="PSUM") as pff_pool:
                    nc.sync.dma_start(masks[0][:], mask0_d[:])
                    nc.sync.dma_start(masks[1][:], mask1_d[:])
                    nc.sync.dma_start(b_in_sb[:], b_in_t[:])
                    nc.sync.dma_start(b_out_sb[:], b_out_t[:])

                    def attn_head(blk, h):
                        sb = blk + 1
                        ft, r0 = h // 2, 64 * (h % 2)
                        sc = sc_pool.tile([128, 2560], F32, tag="sc", name="sc")
                        ex = exp_pool.tile([128, 2560], BF, tag="ex", name="ex")
                        pa = pa_pool.tile([65, 512], F32, tag="pa", name="pa")
                        for half, jts in enumerate((JT_HALF1, JT_HALF2)):
                            for jt in jts:
                                own = jt >= 4
                                b = jt % 4
                                jcol = 512 * (sb if own else sb - 1) + 128 * b
                                q0, q1 = (128 * b, 512) if own else (0, 128 * (b + 1))
                                eo = EXOFF[jt]
                                nc.tensor.matmul(
                                    sc[:, eo:eo + (q1 - q0)],
                                    kT[ft][r0:r0 + 64, jcol:jcol + 128],
                                    qT[ft][r0:r0 + 64,
                                           512 * blk + q0:512 * blk + q1],
                                    start=True, stop=True)
                            c0, c1 = (0, 1536) if half == 0 else (1536, 2560)
                            nc.scalar.activation(
                                ex[:, c0:c1], sc[:, c0:c1],
                                AF.Exp, scale=float(1.0 / np.sqrt(HD)))
                            nc.vector.tensor_mul(ex[:, c0:c1], ex[:, c0:c1],
                                                 masks[blk][:, c0:c1])
                            for idx, jt in enumerate(jts):
                                own = jt >= 4
                                b = jt % 4
                                q0, q1 = (128 * b, 512) if own else (0, 128 * (b + 1))
                                eo = EXOFF[jt]
                                vt = vbuf[sb if own else sb - 1][b]
                                nc.tensor.matmul(
                                    pa[:, q0:q1], vt[:, 65 * h:65 * h + 65],
                                    ex[:, eo:eo + (q1 - q0)],
                                    start=(half == 0 and idx == 0),
                                    stop=(half == 1 and idx == 3))
                        # raw numerator -> attnT (normalized at stage F);
                        # denominator row -> den_sb[h]
                        nc.scalar.copy(
                            attnT[ft][r0:r0 + 64, 512 * blk:512 * (blk + 1)],
                            pa[0:64, :])
                        dt = den_pool.tile([65, 512], F32, tag="dt", name="dt")
                        nc.scalar.copy(dt[64:65, :], pa[64:65, :])
                        nc.gpsimd.dma_start(
                            den_sb[h:h + 1, 512 * blk:512 * (blk + 1)],
                            dt[64:65, :])

                    def ffn_tile(f):
                        wi = wi_pool.tile([128, 1024], BF, tag="wi", name="wi")
                        nc.sync.dma_start(wi[:, :], win_r[f])
                        for c in range(2):
                            ps = pff_pool.tile([128, 512], F32, tag="pff", name="pff")
                            for h in range(8):
                                nc.tensor.matmul(
                                    ps[:], wi[:, 128 * h:128 * (h + 1)],
                                    xbO[h][:, 512 * c:512 * (c + 1)],
                                    start=(h == 0), stop=(h == 7))
                            ffs = ffs_pool.tile([128, 512], BF, tag="ffs", name="ffs")
                            nc.vector.tensor_copy(ffs[:], ps[:])
                            nc.sync.dma_start(
                                ff_d[f, :, 512 * c:512 * (c + 1)], ffs[:])

                    for h in range(NH):
                        attn_head(0, h)
                        ffn_tile(h)
                    for h in range(NH):
                        attn_head(1, h)
                        ffn_tile(16 + h)

            # ---- stage F: output projection (ff-first order) ----
            with tc.tile_pool(name="wof", bufs=10) as wof_pool, \
                 tc.tile_pool(name="ffl", bufs=6) as ffl_pool, \
                 tc.tile_pool(name="ffg", bufs=4) as ffg_pool, \
                 tc.tile_pool(name="osb", bufs=4) as osb_pool, \
                 tc.tile_pool(name="rcb", bufs=2) as rcb_pool, \
                 tc.tile_pool(name="pout", bufs=8, space="PSUM") as pout_pool:
                # batched softmax normalization: one reciprocal, 8 broadcasts
                rden = attnT_pool.tile([16, 1024], BF, tag="rden", name="rden")
                with nc.allow_low_precision(reason="softmax denom recip to bf16"):
                    nc.vector.reciprocal(rden[:], den_sb[:])
                for ft in range(8):
                    rcb = rcb_pool.tile([128, 1024], BF, tag="rcb", name="rcb")
                    nc.gpsimd.dma_start(
                        rcb[:],
                        rden[2 * ft:2 * ft + 2, :].rearrange(
                            "p (o f) -> p o f", o=1).to_broadcast((2, 64, 1024)))
                    nc.vector.tensor_mul(attnT[ft][:], attnT[ft][:], rcb[:])

                forder = list(range(8, 40)) + list(range(8))
                for g in range(2):
                    pso = [[pout_pool.tile([128, 512], F32, tag=f"po{j}{c}",
                                           name=f"po{j}{c}", bufs=1)
                            for c in range(2)] for j in range(4)]
                    wof = None
                    for fi, f in enumerate(forder):
                        if f < 8:
                            comb_f = attnT[f]
                        else:
                            ffl = ffl_pool.tile([128, OWN], BF, tag="ffl",
                                                name="ffl")
                            nc.gpsimd.dma_start(ffl[:], ff_d[f - 8])
                            comb_f = ffg_pool.tile([128, OWN], BF, tag="ffg",
                                                   name="ffg")
                            nc.scalar.activation(comb_f[:], ffl[:], AF.Gelu,
                                                 bias=b_in_sb[:, f - 8:f - 7],
                                                 scale=1.0)
                        if fi % 2 == 0:
                            wof = wof_pool.tile([128, 1024], BF, tag="wof",
                                                name="wof")
                            nc.sync.dma_start(wof[:, :], wout_r[g, f // 2])
                        w0 = 512 * (f % 2)
                        for j in range(4):
                            for c in range(2):
                                nc.tensor.matmul(
                                    pso[j][c][:],
                                    wof[:, w0 + 128 * j:w0 + 128 * (j + 1)],
                                    comb_f[:, 512 * c:512 * (c + 1)],
                                    start=(fi == 0), stop=(fi == 39))
                    for j in range(4):
                        n = 4 * g + j
                        osb = osb_pool.tile([128, OWN], F32, tag="osb", name="osb")
                        for c in range(2):
                            nc.scalar.activation(osb[:, 512 * c:512 * (c + 1)],
                                                 pso[j][c][:], AF.Identity,
                                                 bias=b_out_sb[:, n:n + 1], scale=1.0)
                        nc.sync.dma_start(out_d[128 * n:128 * (n + 1), :], osb[:])

    _split_sync_waits(nc, mybir)
    _NC_CACHE = nc
    return nc


def _build_masks():
    """mask_norm / mask_first: [128, 2560] multiplicative masks over the
    packed score region (partitions = key index within chunk, free = packed
    query columns). mask_first additionally kills all halo chunks (for the
    first q-block of cores with no real halo)."""
    qi = np.arange(128)[:, None]        # key row (partition)
    kq = np.arange(128)[None, :]        # query col
    tri_own = (kq >= qi).astype(np.float32)   # keep q >= k
    tri_halo = (kq < qi).astype(np.float32)   # keep q < k
    norm = np.ones((128, 2560), np.float32)
    first = np.ones((128, 2560), np.float32)
    for jt in range(8):
        eo, ew = EXOFF[jt], EXW[jt]
        if jt >= 4:
            norm[:, eo:eo + 128] = tri_own
            first[:, eo:eo + 128] = tri_own
        else:
            norm[:, eo + ew - 128:eo + ew] = tri_halo
            first[:, eo:eo + ew] = 0.0
    return norm.astype(bf16), first.astype(bf16)


def kernel(x, sin, cos, norm_w, w_qkv, b_qkv, w_in, b_in, w_out, b_out,
           attention_width):
    assert int(attention_width) == W
    from concourse.bass_utils import run_bass_kernel_spmd

    x = np.asarray(x, np.float32)
    sin2 = np.asarray(sin, np.float32)[:, 0, :]      # (S, 64)
    cos2 = np.asarray(cos, np.float32)[:, 0, :]
    norm_w = np.asarray(norm_w, np.float32)
    w_qkv = np.asarray(w_qkv, np.float32) * norm_w[:, None]
    w_in_f = np.asarray(w_in, np.float32) * norm_w[:, None]
    w_out_f = np.asarray(w_out, np.float32)
    b_in_f = np.asarray(b_in, np.float32)
    b_out_f = np.asarray(b_out, np.float32)
    b_qkv = np.asarray(b_qkv, np.float32)
    assert np.all(b_qkv == 0.0), "kernel assumes zero qkv bias"

    wqkv_r = np.ascontiguousarray(
        w_qkv.reshape(8, 128, 24, 128).transpose(2, 1, 0, 3)
        .reshape(24, 128, 1024)[:16]).astype(bf16)
    wv_r = np.ascontiguousarray(
        w_qkv[:, 2048:3072].reshape(8, 128, 1024)).astype(bf16)
    win_r = np.ascontiguousarray(
        w_in_f.reshape(8, 128, 32, 128).transpose(2, 1, 0, 3)
        .reshape(32, 128, 1024)).astype(bf16)
    wout_r = np.ascontiguousarray(
        w_out_f.reshape(20, 2, 128, 2, 4, 128).transpose(3, 0, 2, 1, 4, 5)
        .reshape(2, 20, 128, 1024)).astype(bf16)
    b_in_t = np.ascontiguousarray(b_in_f.reshape(32, 128).T)
    b_out_t = np.ascontiguousarray(b_out_f.reshape(8, 128).T)
    sgn = np.where(np.arange(HD) % 2 == 0, -1.0, 1.0).astype(np.float32)
    mask_norm, mask_first = _build_masks()

    in_maps = []
    for core in range(N_CORES):
        b, c = core // 4, core % 4
        t0 = c * OWN
        xTs = np.zeros((H, SHARD), np.float32)
        sc = np.zeros((SHARD, HD), np.float32)
        cc = np.ones((SHARD, HD), np.float32)
        if c == 0:
            xTs[:, W:] = x[b, t0:t0 + OWN].T
            sc[W:] = sin2[t0:t0 + OWN]
            cc[W:] = cos2[t0:t0 + OWN]
            m0 = mask_first
        else:
            xTs[:, :] = x[b, t0 - W:t0 + OWN].T
            sc[:] = sin2[t0 - W:t0 + OWN]
            cc[:] = cos2[t0 - W:t0 + OWN]
            m0 = mask_norm
        cosT = np.ascontiguousarray(np.tile(cc.T, (2, 1)))
        sinST = np.ascontiguousarray(np.tile((sc * sgn[None, :]).T, (2, 1)))
        in_maps.append({
            "xbT": np.ascontiguousarray(xTs).astype(bf16),
            "cosT": cosT, "sinST": sinST,
            "wqkv_r": wqkv_r, "wv_r": wv_r, "win_r": win_r, "wout_r": wout_r,
            "b_in_t": b_in_t, "b_out_t": b_out_t,
            "mask0": m0, "mask1": mask_norm,
        })

    nc = _build()
    res = run_bass_kernel_spmd(nc, in_maps, core_ids=list(range(N_CORES)))

    out = np.empty((B, S, H), np.float32)
    for core in range(N_CORES):
        b, c = core // 4, core % 4
        out[b, c * OWN:(c + 1) * OWN, :] = res.results[core]["outT"].T
    return out


# revision 40
# speedup vs baseline: 1.0742x; 1.0742x over previous
"""Trainium2 Bass kernel for nn_FEMREncoderLayer (RMSNorm + fused QKV + RoPE +
sliding-window local attention + parallel gelu FFN + joint output projection).

Data-parallel over 8 NeuronCores: core i handles batch i//4, tokens
[(i%4)*1024, (i%4)*1024+1024), with a 512-token halo for the local attention's
previous-block keys/values (zeros + masked out for the first block of each
batch). Q/K flow feature-major so weight tiles are the PE stationary operand;
V is produced token-major directly (no on-device transposes). Attention
scores for one (head, q-block) land in a bin-packed 5-bank PSUM region so the
softmax exp runs as two wide ACT calls; the causal/halo mask is one batched
multiply against a host-built mask tile; softmax denominators are collected
and inverted in a single batched reciprocal at the output-projection stage.
Matmul inputs are bf16 (fp32 PSUM accumulation).
"""
import numpy as np
import ml_dtypes
from contextlib import ExitStack

B, S, H, NH, HD, I, W = 2, 4096, 1024, 16, 64, 4096, 512
EPS = 1e-6
N_CORES = 8
OWN = 1024            # tokens owned per core
SHARD = OWN + W       # plus halo

bf16 = ml_dtypes.bfloat16

# stream_shuffle mask: swap adjacent partition pairs within each 32-group
_SHUF = []
for _i in range(16):
    _SHUF += [2 * _i + 1, 2 * _i]

# score-region packing: per key-chunk jt -> (offset, width) in the [128,2560]
# PSUM region. jt 0-3 = halo 128-chunks, jt 4-7 = own 128-chunks. Widths are
# the valid query ranges; offsets bin-pack into 512-f32 PSUM banks.
EXW = [128, 256, 384, 512, 512, 384, 256, 128]
EXOFF = {4: 0, 3: 512, 5: 1024, 0: 1408, 2: 1536, 7: 1920, 1: 2048, 6: 2304}
JT_HALF1 = [4, 3, 5, 0]   # fill banks 0-2 (cols 0:1536)
JT_HALF2 = [2, 7, 1, 6]   # fill banks 3-4 (cols 1536:2560)

_NC_CACHE = None


def _split_sync_waits(nc, mybir, max_waits=1):
    """This container's walrus encodes at most one sync-wait command per
    instruction; spread Tile's extra waits over preceding same-engine NoOps."""
    for f in nc.m.functions:
        for bb in f.blocks:
            out = []
            changed = False
            for ins in bb.instructions:
                si = ins.sync_info
                if si is not None and si.on_wait and len(si.on_wait) > max_waits:
                    waits = list(si.on_wait)
                    extra, keep = waits[:-max_waits], waits[-max_waits:]
                    for i, w in enumerate(extra):
                        out.append(mybir.InstNoOp(
                            name=f"{ins.name}-sw{i}", engine=ins.engine,
                            ins=[], outs=[],
                            sync_info=mybir.SyncInfo(on_wait=[w], on_update=[])))
                    si.on_wait = keep
                    changed = True
                out.append(ins)
            if changed:
                del bb.instructions[:]
                for ins in out:
                    bb.add_instruction(ins)
    return nc


def _build():
    global _NC_CACHE
    if _NC_CACHE is not None:
        return _NC_CACHE
    import concourse.bass as bass
    import concourse.tile as tile
    import concourse.mybir as mybir

    F32 = mybir.dt.float32
    BF = mybir.dt.bfloat16
    AF = mybir.ActivationFunctionType

    nc = bass.Bass()
    xbT_d = nc.dram_tensor("xbT", [H, SHARD], BF, kind="ExternalInput")
    cosT = nc.dram_tensor("cosT", [128, SHARD], F32, kind="ExternalInput")
    sinST = nc.dram_tensor("sinST", [128, SHARD], F32, kind="ExternalInput")
    wqkv_r = nc.dram_tensor("wqkv_r", [16, 128, 1024], BF, kind="ExternalInput")
    wv_r = nc.dram_tensor("wv_r", [8, 128, 1024], BF, kind="ExternalInput")
    win_r = nc.dram_tensor("win_r", [32, 128, 1024], BF, kind="ExternalInput")
    wout_r = nc.dram_tensor("wout_r", [2, 20, 128, 1024], BF, kind="ExternalInput")
    b_in_t = nc.dram_tensor("b_in_t", [128, 32], F32, kind="ExternalInput")
    b_out_t = nc.dram_tensor("b_out_t", [128, 8], F32, kind="ExternalInput")
    mask0_d = nc.dram_tensor("mask0", [128, 2560], BF, kind="ExternalInput")
    mask1_d = nc.dram_tensor("mask1", [128, 2560], BF, kind="ExternalInput")
    out_d = nc.dram_tensor("outT", [H, OWN], F32, kind="ExternalOutput")
    ff_d = nc.dram_tensor("ff_d", [32, 128, OWN], BF, kind="Internal")
    rs_d = nc.dram_tensor("rs_sc", [12, 128], F32, kind="Internal")
    rs2_d = nc.dram_tensor("rs_sc2", [12, 128], BF, kind="Internal")

    with tile.TileContext(nc) as tc, ExitStack() as top:
        consts = top.enter_context(tc.tile_pool(name="consts", bufs=1))
        masks = [consts.tile([128, 2560], BF, tag=f"mask{b}", name=f"mask{b}")
                 for b in range(2)]
        ones128 = consts.tile([128, 1], BF, tag="ones128", name="ones128")
        b_in_sb = consts.tile([128, 32], F32, tag="b_in_sb", name="b_in_sb")
        b_out_sb = consts.tile([128, 8], F32, tag="b_out_sb", name="b_out_sb")
        eps_sb = consts.tile([1, 1], F32, tag="eps_sb", name="eps_sb")
        rsB2 = consts.tile([128, SHARD], BF, tag="rsB2", name="rsB2")
        rsT = consts.tile([128, 12], F32, tag="rsT", name="rsT")
        nc.vector.memset(eps_sb[:], EPS)
        nc.vector.memset(ones128[:], 1.0)

        attnT_pool = top.enter_context(tc.tile_pool(name="attnT", bufs=1))
        attnT = [attnT_pool.tile([128, OWN], BF, tag=f"at{i}", name=f"at{i}")
                 for i in range(8)]
        den_sb = attnT_pool.tile([16, 1024], F32, tag="den_sb", name="den_sb")

        bd = ExitStack()
        with bd:
            # ---- stages A-C live inside this scope; its SBUF frees before F
            with tc.tile_pool(name="qT", bufs=1) as qT_pool, \
                 tc.tile_pool(name="kT", bufs=1) as kT_pool, \
                 tc.tile_pool(name="vb", bufs=1) as vb_pool, \
                 tc.tile_pool(name="xbo", bufs=1) as xbo_pool:
                qT = [qT_pool.tile([128, OWN], BF, tag=f"q{i}", name=f"q{i}")
                      for i in range(8)]
                kT = [kT_pool.tile([128, SHARD], BF, tag=f"k{i}", name=f"k{i}")
                      for i in range(8)]
                vbuf = [[vb_pool.tile([128, 16 * 65], BF, tag=f"vb{p}{k}",
                                      name=f"vb{p}{k}") for k in range(4)]
                        for p in range(3)]
                for p in range(3):
                    for k in range(4):
                        ap = vbuf[p][k][:, :].rearrange(
                            "p (h s) -> p h s", s=65)[:, :, 64:65]
                        nc.vector.memset(ap, 1.0)
                xbO = [xbo_pool.tile([128, OWN], BF, tag=f"xb{i}", name=f"xb{i}")
                       for i in range(8)]

                # ---- stage A+B: x load, stats, QKV+RoPE, token-major V ----
                with tc.tile_pool(name="xbh", bufs=1) as xbh_pool, \
                     tc.tile_pool(name="wvp", bufs=1) as wv_pool:
                    xbH = [xbh_pool.tile([128, W], BF, tag=f"xh{i}", name=f"xh{i}")
                           for i in range(8)]
                    wv_sb = [wv_pool.tile([128, 1024], BF, tag=f"wv{h}",
                                          name=f"wv{h}") for h in range(8)]
                    with tc.tile_pool(name="aux", bufs=1) as aux_pool, \
                         tc.tile_pool(name="sq", bufs=2) as sq_pool, \
                         tc.tile_pool(name="wq", bufs=3) as wq_pool, \
                         tc.tile_pool(name="rsc", bufs=3) as rsc_pool, \
                         tc.tile_pool(name="pms", bufs=1, space="PSUM") as pms_pool, \
                         tc.tile_pool(name="pqkv", bufs=4, space="PSUM") as pqkv_pool:
                        cosR = aux_pool.tile([128, SHARD], F32, tag="cosR", name="cosR")
                        sinR = aux_pool.tile([128, SHARD], F32, tag="sinR", name="sinR")
                        rs_tmp = aux_pool.tile([1, SHARD], F32, tag="rs_tmp",
                                               name="rs_tmp")
                        nc.sync.dma_start(cosR[:], cosT[:])
                        nc.sync.dma_start(sinR[:], sinST[:])
                        for i in range(8):
                            nc.sync.dma_start(xbH[i][:],
                                              xbT_d[128 * i:128 * (i + 1), 0:W])
                            nc.sync.dma_start(xbO[i][:],
                                              xbT_d[128 * i:128 * (i + 1), W:])
                        pms = pms_pool.tile([1, SHARD], F32, tag="pms", name="pms")
                        for i in range(8):
                            sqh = sq_pool.tile([128, W], BF, tag="sqh", name="sqh")
                            nc.scalar.square(sqh[:], xbH[i][:])
                            nc.tensor.matmul(pms[:, 0:512], ones128[:], sqh[:],
                                             start=(i == 0), stop=(i == 7))
                            sqo = sq_pool.tile([128, OWN], BF, tag="sqo", name="sqo")
                            nc.scalar.square(sqo[:], xbO[i][:])
                            for c in range(2):
                                nc.tensor.matmul(
                                    pms[:, 512 * (c + 1):512 * (c + 2)],
                                    ones128[:], sqo[:, 512 * c:512 * (c + 1)],
                                    start=(i == 0), stop=(i == 7))
                        # rs = 1/sqrt(ms/H + eps), then x *= rs in place so
                        # Q/K/V/FFN matmuls all consume the normalized x.
                        # The reciprocal runs on the [128,12] token-major view
                        # (DVE recip on 1 partition costs ~13us; on 128 it's
                        # ~0.1us). SBUF->SBUF partition-remap DMAs produce
                        # garbage, so the remaps round-trip through DRAM.
                        nc.scalar.activation(rs_tmp[:], pms[:], AF.Sqrt,
                                             bias=eps_sb[:], scale=1.0 / H)
                        nc.sync.dma_start(rs_d[:, :], rs_tmp[:, :])
                        nc.sync.dma_start(
                            rsT[:, :], rs_d[:, :].rearrange("j p -> p j"))
                        rsTb = aux_pool.tile([128, 12], BF, tag="rsTb",
                                             name="rsTb")
                        with nc.allow_low_precision(reason="rms scale to bf16"):
                            nc.vector.reciprocal(rsTb[:, :], rsT[:, :])
                        nc.sync.dma_start(
                            rs2_d[:, :].rearrange("j p -> p j"), rsTb[:, :])
                        nc.sync.dma_start(
                            rsB2[:],
                            rs2_d[:, :].rearrange("j p -> (j p)").rearrange(
                                "(o t) -> o t", o=1).to_broadcast((128, SHARD)))
                        for i in range(8):
                            nc.vector.tensor_mul(xbH[i][:], xbH[i][:],
                                                 rsB2[:, 0:W])
                            nc.vector.tensor_mul(xbO[i][:], xbO[i][:],
                                                 rsB2[:, W:SHARD])

                        # Q (m 0-7) and K (m 8-15) with RoPE fused from PSUM
                        for m in range(16):
                            is_q = m < 8
                            chunks = (1, 2) if is_q else (0, 1, 2)
                            wqm = wq_pool.tile([128, 1024], BF, tag="wqm", name="wqm")
                            nc.sync.dma_start(wqm[:, :], wqkv_r[m])
                            for c in chunks:
                                ps = pqkv_pool.tile([128, 512], F32, tag="pqkv",
                                                    name="pqkv")
                                for h in range(8):
                                    rhs = (xbH[h][:] if c == 0
                                           else xbO[h][:, 512 * (c - 1):512 * c])
                                    nc.tensor.matmul(
                                        ps[:], wqm[:, 128 * h:128 * (h + 1)],
                                        rhs, start=(h == 0), stop=(h == 7))
                                if is_q:
                                    dest = qT[m][:, 512 * (c - 1):512 * c]
                                else:
                                    dest = kT[m - 8][:, 512 * c:512 * (c + 1)]
                                qc = rsc_pool.tile([128, 512], BF, tag="qc", name="qc")
                                shf = rsc_pool.tile([128, 512], F32, tag="shf",
                                                    name="shf")
                                shs = rsc_pool.tile([128, 512], BF, tag="shs",
                                                    name="shs")
                                nc.vector.tensor_mul(
                                    qc[:], ps[:], cosR[:, 512 * c:512 * (c + 1)])
                                nc.vector.stream_shuffle(shf[:], ps[:], _SHUF)
                                nc.gpsimd.tensor_mul(
                                    shs[:], shf[:], sinR[:, 512 * c:512 * (c + 1)])
                                nc.gpsimd.tensor_add(dest, qc[:], shs[:])

                    # V: token-major (stationary = normalized x, moving = w_v)
                    with tc.tile_pool(name="pv", bufs=4, space="PSUM") as pv_pool:
                        for h in range(8):
                            nc.sync.dma_start(wv_sb[h][:], wv_r[h])
                        for T in range(12):
                            t0 = 128 * T if T < 4 else 128 * (T - 4)
                            for c in range(2):
                                psv = pv_pool.tile([128, 512], F32, tag="psv",
                                                   name="psv")
                                for h in range(8):
                                    xst = (xbH[h][:, t0:t0 + 128] if T < 4
                                           else xbO[h][:, t0:t0 + 128])
                                    nc.tensor.matmul(
                                        psv[:], xst,
                                        wv_sb[h][:, 512 * c:512 * (c + 1)],
                                        start=(h == 0), stop=(h == 7))
                                dst = vbuf[T // 4][T % 4][:, :].rearrange(
                                    "p (h s) -> p h s", s=65)[:, 8 * c:8 * c + 8, 0:64]
                                src = psv[:, :].rearrange("p (h s) -> p h s", s=64)
                                nc.vector.tensor_copy(dst, src)

                # ---- stage C: attention + FFN, interleaved ----
                with tc.tile_pool(name="exp", bufs=2) as exp_pool, \
                     tc.tile_pool(name="wi", bufs=3) as wi_pool, \
                     tc.tile_pool(name="ffs", bufs=4) as ffs_pool, \
                     tc.tile_pool(name="dent", bufs=2) as den_pool, \
                     tc.tile_pool(name="sc", bufs=1, space="PSUM") as sc_pool, \
                     tc.tile_pool(name="pa", bufs=2, space="PSUM") as pa_pool, \
                     tc.tile_pool(name="pff", bufs=1, space="PSUM") as pff_pool:
                    nc.sync.dma_start(masks[0][:], mask0_d[:])
                    nc.sync.dma_start(masks[1][:], mask1_d[:])
                    nc.sync.dma_start(b_in_sb[:], b_in_t[:])
                    nc.sync.dma_start(b_out_sb[:], b_out_t[:])

                    def attn_head(blk, h):
                        sb = blk + 1
                        ft, r0 = h // 2, 64 * (h % 2)
                        sc = sc_pool.tile([128, 2560], F32, tag="sc", name="sc")
                        ex = exp_pool.tile([128, 2560], BF, tag="ex", name="ex")
                        pa = pa_pool.tile([65, 512], F32, tag="pa", name="pa")
                        for half, jts in enumerate((JT_HALF1, JT_HALF2)):
                            for jt in jts:
                                own = jt >= 4
                                b = jt % 4
                                jcol = 512 * (sb if own else sb - 1) + 128 * b
                                q0, q1 = (128 * b, 512) if own else (0, 128 * (b + 1))
                                eo = EXOFF[jt]
                                nc.tensor.matmul(
                                    sc[:, eo:eo + (q1 - q0)],
                                    kT[ft][r0:r0 + 64, jcol:jcol + 128],
                                    qT[ft][r0:r0 + 64,
                                           512 * blk + q0:512 * blk + q1],
                                    start=True, stop=True)
                            c0, c1 = (0, 1536) if half == 0 else (1536, 2560)
                            nc.scalar.activation(
                                ex[:, c0:c1], sc[:, c0:c1],
                                AF.Exp, scale=float(1.0 / np.sqrt(HD)))
                            nc.vector.tensor_mul(ex[:, c0:c1], ex[:, c0:c1],
                                                 masks[blk][:, c0:c1])
                            for idx, jt in enumerate(jts):
                                own = jt >= 4
                                b = jt % 4
                                q0, q1 = (128 * b, 512) if own else (0, 128 * (b + 1))
                                eo = EXOFF[jt]
                                vt = vbuf[sb if own else sb - 1][b]
                                nc.tensor.matmul(
                                    pa[:, q0:q1], vt[:, 65 * h:65 * h + 65],
                                    ex[:, eo:eo + (q1 - q0)],
                                    start=(half == 0 and idx == 0),
                                    stop=(half == 1 and idx == 3))
                        # raw numerator -> attnT (normalized at stage F);
                        # denominator row -> den_sb[h]
                        nc.vector.tensor_copy(
                            attnT[ft][r0:r0 + 64, 512 * blk:512 * (blk + 1)],
                            pa[0:64, :])
                        dt = den_pool.tile([65, 512], F32, tag="dt", name="dt")
                        nc.vector.tensor_copy(dt[64:65, :], pa[64:65, :])
                        nc.gpsimd.dma_start(
                            den_sb[h:h + 1, 512 * blk:512 * (blk + 1)],
                            dt[64:65, :])

                    def ffn_tile(f):
                        wi = wi_pool.tile([128, 1024], BF, tag="wi", name="wi")
                        nc.sync.dma_start(wi[:, :], win_r[f])
                        for c in range(2):
                            ps = pff_pool.tile([128, 512], F32, tag="pff", name="pff")
                            for h in range(8):
                                nc.tensor.matmul(
                                    ps[:], wi[:, 128 * h:128 * (h + 1)],
                                    xbO[h][:, 512 * c:512 * (c + 1)],
                                    start=(h == 0), stop=(h == 7))
                            ffs = ffs_pool.tile([128, 512], BF, tag="ffs", name="ffs")
                            nc.vector.tensor_copy(ffs[:], ps[:])
                            nc.sync.dma_start(
                                ff_d[f, :, 512 * c:512 * (c + 1)], ffs[:])

                    for h in range(NH):
                        attn_head(0, h)
                        ffn_tile(h)
                    for h in range(NH):
                        attn_head(1, h)
                        ffn_tile(16 + h)

            # ---- stage F: output projection (ff-first order) ----
            with tc.tile_pool(name="wof", bufs=10) as wof_pool, \
                 tc.tile_pool(name="ffl", bufs=6) as ffl_pool, \
                 tc.tile_pool(name="ffg", bufs=4) as ffg_pool, \
                 tc.tile_pool(name="osb", bufs=4) as osb_pool, \
                 tc.tile_pool(name="rcb", bufs=2) as rcb_pool, \
                 tc.tile_pool(name="pout", bufs=8, space="PSUM") as pout_pool:
                # batched softmax normalization: one reciprocal, 8 broadcasts
                rden = attnT_pool.tile([16, 1024], BF, tag="rden", name="rden")
                with nc.allow_low_precision(reason="softmax denom recip to bf16"):
                    nc.vector.reciprocal(rden[:], den_sb[:])
                for ft in range(8):
                    rcb = rcb_pool.tile([128, 1024], BF, tag="rcb", name="rcb")
                    nc.gpsimd.dma_start(
                        rcb[:],
                        rden[2 * ft:2 * ft + 2, :].rearrange(
                            "p (o f) -> p o f", o=1).to_broadcast((2, 64, 1024)))
                    nc.vector.tensor_mul(attnT[ft][:], attnT[ft][:], rcb[:])

                forder = list(range(8, 40)) + list(range(8))
                for g in range(2):
                    pso = [[pout_pool.tile([128, 512], F32, tag=f"po{j}{c}",
                                           name=f"po{j}{c}", bufs=1)
                            for c in range(2)] for j in range(4)]
                    wof = None
                    for fi, f in enumerate(forder):
                        if f < 8:
                            comb_f = attnT[f]
                        else:
                            ffl = ffl_pool.tile([128, OWN], BF, tag="ffl",
                                                name="ffl")
                            nc.sync.dma_start(ffl[:], ff_d[f - 8])
                            comb_f = ffg_pool.tile([128, OWN], BF, tag="ffg",
                                                   name="ffg")
                            nc.scalar.activation(comb_f[:], ffl[:], AF.Gelu,
                                                 bias=b_in_sb[:, f - 8:f - 7],
                                                 scale=1.0)
                        if fi % 2 == 0:
                            wof = wof_pool.tile([128, 1024], BF, tag="wof",
                                                name="wof")
                            nc.sync.dma_start(wof[:, :], wout_r[g, f // 2])
                        w0 = 512 * (f % 2)
                        for j in range(4):
                            for c in range(2):
                                nc.tensor.matmul(
                                    pso[j][c][:],
                                    wof[:, w0 + 128 * j:w0 + 128 * (j + 1)],
                                    comb_f[:, 512 * c:512 * (c + 1)],
                                    start=(fi == 0), stop=(fi == 39))
                    for j in range(4):
                        n = 4 * g + j
                        osb = osb_pool.tile([128, OWN], F32, tag="osb", name="osb")
                        for c in range(2):
                            nc.scalar.activation(osb[:, 512 * c:512 * (c + 1)],
                                                 pso[j][c][:], AF.Identity,
                                                 bias=b_out_sb[:, n:n + 1], scale=1.0)
                        nc.sync.dma_start(out_d[128 * n:128 * (n + 1), :], osb[:])

    _split_sync_waits(nc, mybir)
    _NC_CACHE = nc
    return nc


def _build_masks():
    """mask_norm / mask_first: [128, 2560] multiplicative masks over the
    packed score region (partitions = key index within chunk, free = packed
    query columns). mask_first additionally kills all halo chunks (for the
    first q-block of cores with no real halo)."""
    qi = np.arange(128)[:, None]        # key row (partition)
    kq = np.arange(128)[None, :]        # query col
    tri_own = (kq >= qi).astype(np.float32)   # keep q >= k
    tri_halo = (kq < qi).astype(np.float32)   # keep q < k
    norm = np.ones((128, 2560), np.float32)
    first = np.ones((128, 2560), np.float32)
    for jt in range(8):
        eo, ew = EXOFF[jt], EXW[jt]
        if jt >= 4:
            norm[:, eo:eo + 128] = tri_own
            first[:, eo:eo + 128] = tri_own
        else:
            norm[:, eo + ew - 128:eo + ew] = tri_halo
            first[:, eo:eo + ew] = 0.0
    return norm.astype(bf16), first.astype(bf16)


def kernel(x, sin, cos, norm_w, w_qkv, b_qkv, w_in, b_in, w_out, b_out,
           attention_width):
    assert int(attention_width) == W
    from concourse.bass_utils import run_bass_kernel_spmd

    x = np.asarray(x, np.float32)
    sin2 = np.asarray(sin, np.float32)[:, 0, :]      # (S, 64)
    cos2 = np.asarray(cos, np.float32)[:, 0, :]
    norm_w = np.asarray(norm_w, np.float32)
    w_qkv = np.asarray(w_qkv, np.float32) * norm_w[:, None]
    w_in_f = np.asarray(w_in, np.float32) * norm_w[:, None]
    w_out_f = np.asarray(w_out, np.float32)
    b_in_f = np.asarray(b_in, np.float32)
    b_out_f = np.asarray(b_out, np.float32)
    b_qkv = np.asarray(b_qkv, np.float32)
    assert np.all(b_qkv == 0.0), "kernel assumes zero qkv bias"

    wqkv_r = np.ascontiguousarray(
        w_qkv.reshape(8, 128, 24, 128).transpose(2, 1, 0, 3)
        .reshape(24, 128, 1024)[:16]).astype(bf16)
    wv_r = np.ascontiguousarray(
        w_qkv[:, 2048:3072].reshape(8, 128, 1024)).astype(bf16)
    win_r = np.ascontiguousarray(
        w_in_f.reshape(8, 128, 32, 128).transpose(2, 1, 0, 3)
        .reshape(32, 128, 1024)).astype(bf16)
    wout_r = np.ascontiguousarray(
        w_out_f.reshape(20, 2, 128, 2, 4, 128).transpose(3, 0, 2, 1, 4, 5)
        .reshape(2, 20, 128, 1024)).astype(bf16)
    b_in_t = np.ascontiguousarray(b_in_f.reshape(32, 128).T)
    b_out_t = np.ascontiguousarray(b_out_f.reshape(8, 128).T)
    sgn = np.where(np.arange(HD) % 2 == 0, -1.0, 1.0).astype(np.float32)
    mask_norm, mask_first = _build_masks()

    in_maps = []
    for core in range(N_CORES):
        b, c = core // 4, core % 4
        t0 = c * OWN
        xTs = np.zeros((H, SHARD), np.float32)
        sc = np.zeros((SHARD, HD), np.float32)
        cc = np.ones((SHARD, HD), np.float32)
        if c == 0:
            xTs[:, W:] = x[b, t0:t0 + OWN].T
            sc[W:] = sin2[t0:t0 + OWN]
            cc[W:] = cos2[t0:t0 + OWN]
            m0 = mask_first
        else:
            xTs[:, :] = x[b, t0 - W:t0 + OWN].T
            sc[:] = sin2[t0 - W:t0 + OWN]
            cc[:] = cos2[t0 - W:t0 + OWN]
            m0 = mask_norm
        cosT = np.ascontiguousarray(np.tile(cc.T, (2, 1)))
        sinST = np.ascontiguousarray(np.tile((sc * sgn[None, :]).T, (2, 1)))
        in_maps.append({
            "xbT": np.ascontiguousarray(xTs).astype(bf16),
            "cosT": cosT, "sinST": sinST,
            "wqkv_r": wqkv_r, "wv_r": wv_r, "win_r": win_r, "wout_r": wout_r,
            "b_in_t": b_in_t, "b_out_t": b_out_t,
            "mask0": m0, "mask1": mask_norm,
        })

    nc = _build()
    res = run_bass_kernel_spmd(nc, in_maps, core_ids=list(range(N_CORES)))

    out = np.empty((B, S, H), np.float32)
    for core in range(N_CORES):
        b, c = core // 4, core % 4
        out[b, c * OWN:(c + 1) * OWN, :] = res.results[core]["outT"].T
    return out
